# revision 18
# baseline (speedup 1.0000x reference)
"""MQA attention kernel for Trainium2, 8-core SPMD (v5).

Problem: Q [2, 8, 2048, 64] fp32, K/V [2, 1, 2048, 64] fp32 (shared head).
out[b,h,q,:] = softmax(Q[b,h,q,:] @ K[b,0]^T / 8) @ V[b,0].

Sharding: 16 (b,h) pairs over 8 cores -> core c handles b = c//4,
heads 2*(c%4), 2*(c%4)+1 (both heads share one K/V slice).

Design notes:
  - "Permuted-s" staging: all inputs load as Xn[p, c, ...] = X[s=16p+c]
    (128 x 4KB contiguous descriptors per DMA, ~20x fewer than a
    128-partition-tile layout). Queries and keys are processed in this
    permuted order everywhere on-chip (softmax is order-invariant; V uses
    the same key order), and only the final output DMA un-permutes.
  - Zero PE transposes / zero SWDGE / zero DRAM scratch: fp32 HWDGE
    loads, fp16 casts on ACT+DVE (idle in the prologue), QT/KT [128, S]
    via merged SBUF->SBUF XBAR transpose-DMAs (3D-out = chunked
    partition transposes; K is cast column-duplicated so K^T lands
    replicated on both partition halves for free).
  - MM1 (scores^T): per k-tile, two concurrent row-group matmuls
    (contract=64; h0 rows 0-63, h1 rows 64-127) -> ps_s [128k, 2, 512q].
  - exp split: ACT true exp for h0; DVE Schraudolph exp for h1 in one
    tensor_scalar: int16(round(score*A+B)) bitcast fp16 ~= exp(score/8-C)
    (the -C shift cancels in softmax).
  - MM2: out^T[128, 512] += V_aug[kt]^T @ P^T; V_aug = [V | ones | 0-pad
    to 128 cols]. Col 64 accumulates the softmax denominator.
  - Drain (per qb, h): PSUM->SBUF fp16 copy rows 0:80 (ACT h0 / DVE h1),
    ONE merged XBAR transpose [80,512]->[128,4,80], DVE reciprocal of
    col 64, DVE per-chunk tensor_scalar normalize -> fp32, DMA out.
    Drain is EMITTED INTERLEAVED into the next qb's kt loop; all drain
    pools are 4-deep so a slow stage never backpressures the loop.
  - Main loop software-pipelined with MM2 trailing MM1 by 3 k-tiles so
    the PE FIFO never waits on exp; ps_s/ps_o double-buffered = 8 banks.
  - Two PE warmup phases bridge the staging window so HAM is at K=8/8
    when the main loop starts.
"""

import numpy as np

import concourse.bass as bass
import concourse.bacc as bacc
import concourse.mybir as mybir
import concourse.tile as tile
from concourse.bass_utils import run_bass_kernel_spmd

F32 = mybir.dt.float32
F16 = mybir.dt.float16
I16 = mybir.dt.int16

B, H, S, D = 2, 8, 2048, 64
HPC = 2            # heads per core
NCORES = 8
QB = 512           # query block (PSUM bank free-dim limit for fp32)
NQB = S // QB      # 4
KT_TILE = 128      # keys per k-tile (PE contract partition limit)
NKT = S // KT_TILE # 16
NC = NKT // NQB    # 4 c-chunks per qb block
MO = 80            # drained rows: 64 V + 1 denom + 15 pad (16 | 80)
VW = 128           # V_aug weight cols
SCALE = 1.0 / np.sqrt(np.float32(D))  # 0.125
C_SHIFT = 2.0      # exp(z - C_SHIFT): bounds p in fp16/int16; cancels in softmax

# Schraudolph fp16 exp: i16 = round(score*A2 + B2); bitcast fp16 ~= exp(score/8 - C)
A_FP16 = 1024.0 / np.log(2.0)              # 2^10 / ln 2
A2 = float(SCALE) * A_FP16                 # folded score scale
B2 = 15.0 * 1024.0 - C_SHIFT * A_FP16      # exponent bias - shift
MM2_SKEW = 2       # effective skew = MM2_SKEW + 1


def build_nc():
    nc = bacc.Bacc(None)
    Qd = nc.declare_dram_parameter("q", [HPC, S, D], F32, isOutput=False)
    Kd = nc.declare_dram_parameter("k", [S, D], F32, isOutput=False)
    Vd = nc.declare_dram_parameter("v", [S, D], F32, isOutput=False)
    Od = nc.declare_dram_parameter("o", [HPC, S, D], F32, isOutput=True)

    with tile.TileContext(nc) as tc:
        with (
            tc.tile_pool(name="const", bufs=1) as constp,
            tc.tile_pool(name="stage", bufs=1) as stp,
            tc.tile_pool(name="qk", bufs=1) as qkp,
            tc.tile_pool(name="vt", bufs=1) as vp,
            tc.tile_pool(name="pt", bufs=6) as ptp,
            tc.tile_pool(name="otF", bufs=2) as otFp,
            tc.tile_pool(name="otT", bufs=4) as otTp,
            tc.tile_pool(name="rec", bufs=4) as recp,
            tc.tile_pool(name="outsb", bufs=4) as outp,
            tc.tile_pool(name="psS", bufs=2, space="PSUM") as psSp,
            tc.tile_pool(name="psO", bufs=2, space="PSUM") as psOp,
        ):
            # Prime the exp table load so the ~2.7us ACT_TABLE_LOAD overlaps
            # the input DMA phase instead of stalling the first real exp.
            dummy = constp.tile([128, 64], F16)
            nc.vector.memset(dummy[:], 0.0)
            dummy32 = constp.tile([128, 16], F32)
            nc.vector.memset(dummy32[:], 0.0)
            nc.scalar.activation(
                dummy32[:], dummy32[:], mybir.ActivationFunctionType.Exp
            )

            # per-partition bias AP for the ACT exp (const-AP registry only
            # carries pre-registered values)
            bias_ap = constp.tile([128, 1], F32)
            nc.vector.memset(bias_ap[:], -float(C_SHIFT))

            # warmup output scribbles into qb0's accumulator (cleared by the
            # start=True MM2 later)
            ps_o0 = [psOp.tile([128, QB], F32, name=f"psO{h}") for h in range(HPC)]

            # PE warmup phase 1: unblocked, runs as soon as the PE is free.
            for _ in range(68):
                nc.tensor.matmul(
                    ps_o0[0][0:64, 0:64],
                    lhsT=dummy[:, 0:64],
                    rhs=dummy[:],
                    start=True,
                    stop=True,
                )

            # ---- input staging: contiguous permuted-s loads, chunked fp16
            # casts, interleaved merged XBAR transposes ----
            # loads go on the Scalar-engine HWDGE queue so the XBAR
            # transposes (Sync queue) don't wait on the shared per-queue DMA
            # completion counter of unrelated loads
            Kn = stp.tile([128, NKT, D], F32, name="Kn")
            nc.scalar.dma_start(
                out=Kn[:], in_=Kd.ap().rearrange("(p c) d -> p c d", p=128)
            )
            Qn = stp.tile([128, NKT, HPC, D], F32, name="Qn")
            for h in range(HPC):
                nc.scalar.dma_start(
                    out=Qn[:, :, h, :],
                    in_=Qd.ap()[h].rearrange("(p c) d -> p c d", p=128),
                )
            Vn = stp.tile([128, NKT, D], F32, name="Vn")
            nc.scalar.dma_start(
                out=Vn[:], in_=Vd.ap().rearrange("(p c) d -> p c d", p=128)
            )

            # K cast duplicated into both 64-col halves (the XBAR consumes
            # 128-wide input column tiles; the duplicate lands K^T on both
            # partition halves of KT).
            Kh = stp.tile([128, NKT, 2, D], F16, name="Kh")
            Qh = stp.tile([128, NKT, HPC, D], F16, name="Qh")
            for c in range(NQB):
                cs = slice(c * NC, (c + 1) * NC)
                nc.vector.tensor_copy(Kh[:, cs, 0, :], Kn[:, cs, :])
                nc.vector.tensor_copy(Kh[:, cs, 1, :], Kn[:, cs, :])
                nc.scalar.copy(Qh[:, cs, :, :], Qn[:, cs, :, :])

            # V_aug [128k, c, 128] fp16: cols 0-63 = V (same permuted key
            # order as KT), col 64 = 1.0 (denominator), cols 65-127 = 0.
            Vt = vp.tile([128, NKT, VW], F16)
            nc.vector.memset(Vt[:, :, D + 1 : VW], 0.0)
            nc.vector.memset(Vt[:, :, D : D + 1], 1.0)
            nc.scalar.copy(Vt[:, :, 0:D], Vn[:])

            # PE warmup phase 2: gated on the first Q cast so it bridges the
            # cast/XBAR window right up to the main loop.
            for _ in range(40):
                nc.tensor.matmul(
                    ps_o0[0][0:64, 0:128],
                    lhsT=dummy[:, 0:64],
                    rhs=Qh[:, 0, :, :].rearrange("p h d -> p (h d)"),
                    start=True,
                    stop=True,
                )

            # KT/QT [128, c, 128] fp16 via interleaved chunked XBAR
            # transposes: col (c, p) of KT/QT is key/query s = 16p + c.
            KT = qkp.tile([128, NKT, 128], F16, name="KT")
            QT = qkp.tile([128, NKT, 128], F16, name="QT")
            for c in range(NQB):
                cs = slice(c * NC, (c + 1) * NC)
                nc.sync.dma_start(
                    out=KT[:, cs, :],
                    in_=Kh[:, cs, :, :].rearrange("p c r d -> p (c r d)"),
                    transpose=True,
                )
                nc.sync.dma_start(
                    out=QT[:, cs, :],
                    in_=Qh[:, cs, :, :].rearrange("p c h d -> p (c h d)"),
                    transpose=True,
                )

            def QTs(h, qb):
                # [64, 512] moving operand for head h, query block qb
                return (
                    QT[64 * h : 64 * (h + 1), :, :]
                    .rearrange("p c s -> p (c s)")[:, qb * QB : (qb + 1) * QB]
                )

            def KTs(h, kt):
                return KT[64 * h : 64 * (h + 1), kt, :]

            # ---- main loop; the previous qb's drain is emitted interleaved
            # into this qb's kt loop ----
            def drain_stages(qb, ps_o):
                """Finely staged so no single loop step absorbs more than
                ~400ns of extra ACT/DVE work."""
                st = {"otF": [None, None], "otT": [None, None], "rec": [None, None],
                      "out": [None, None]}
                HB = QB // 2

                def s_copy(h, half):
                    def f():
                        if half == 0:
                            st["otF"][h] = otFp.tile([MO, QB], F16, name=f"otF{h}")
                        sl = slice(half * HB, (half + 1) * HB)
                        if h == 0:
                            nc.scalar.copy(st["otF"][h][:, sl], ps_o[h][0:MO, sl])
                        else:
                            nc.vector.tensor_copy(
                                st["otF"][h][:, sl], ps_o[h][0:MO, sl]
                            )
                    return f

                def s_transp(h):
                    def f():
                        st["otT"][h] = otTp.tile(
                            [128, NC, MO], F16, name=f"otT{h}"
                        )
                        nc.sync.dma_start(
                            out=st["otT"][h][:], in_=st["otF"][h][:], transpose=True
                        )
                    return f

                def s_recip(h):
                    def f():
                        otT = st["otT"][h]
                        rec = recp.tile([128, NC, 1], F32)
                        nc.vector.reciprocal(rec[:], otT[:, :, D : D + 1])
                        st["rec"][h] = rec
                        st["out"][h] = outp.tile([128, NC, D], F32, name=f"osb{h}")
                        for j in range(NC // 2):
                            nc.vector.tensor_scalar_mul(
                                st["out"][h][:, j, :], otT[:, j, 0:D], rec[:, j, :]
                            )
                    return f

                def s_norm(h):
                    def f():
                        otT, rec, outsb = st["otT"][h], st["rec"][h], st["out"][h]
                        for j in range(NC // 2, NC):
                            nc.vector.tensor_scalar_mul(
                                outsb[:, j, :], otT[:, j, 0:D], rec[:, j, :]
                            )
                        # un-permute: otT row pp, chunk j -> q = 16*pp + 4*qb + j
                        nc.sync.dma_start(
                            out=Od.ap()[h]
                            .rearrange("(p c) d -> p c d", p=128)[
                                :, qb * NC : (qb + 1) * NC, :
                            ],
                            in_=outsb[:],
                        )
                    return f

                return [
                    s_copy(0, 0), s_copy(0, 1), s_copy(1, 0), s_copy(1, 1),
                    s_transp(0), s_transp(1),
                    s_recip(0), s_norm(0), s_recip(1), s_norm(1),
                ]

            pending_drain = []
            for qb in range(NQB):
                ps_o = (
                    ps_o0
                    if qb == 0
                    else [psOp.tile([128, QB], F32, name=f"psO{h}") for h in range(HPC)]
                )
                pend = []  # software pipeline: MM2 trails MM1 by MM2_SKEW+1 kts
                for kt in range(NKT + MM2_SKEW + 1):
                    if kt < NKT:
                        ps_s = psSp.tile([128, HPC, QB], F32)
                        for h in range(HPC):
                            nc.tensor.matmul(
                                ps_s[:, h, :],
                                lhsT=KTs(h, kt),
                                rhs=QTs(h, qb),
                                start=True,
                                stop=True,
                            )
                    if len(pend) > (MM2_SKEW if kt < NKT else 0):
                        pkt, p0, p1 = pend.pop(0)
                        for h, rhs in ((0, p0[:]), (1, p1[:].bitcast(F16))):
                            nc.tensor.matmul(
                                ps_o[h][:],
                                lhsT=Vt[:, pkt, :],
                                rhs=rhs,
                                start=(pkt == 0),
                                stop=(pkt == NKT - 1),
                            )
                    if pending_drain and kt >= 1:
                        pending_drain.pop(0)()
                    if kt < NKT:
                        # exp: ACT (true) for h0, DVE (Schraudolph) for h1
                        pt0 = ptp.tile([128, QB], F16, name="pt0")
                        nc.scalar.activation(
                            pt0[:],
                            ps_s[:, 0, :],
                            mybir.ActivationFunctionType.Exp,
                            scale=float(SCALE),
                            bias=bias_ap[:],
                        )
                        pt1 = ptp.tile([128, QB], I16, name="pt1")
                        nc.vector.tensor_scalar(
                            pt1[:],
                            ps_s[:, 1, :],
                            float(A2),
                            float(B2),
                            op0=mybir.AluOpType.mult,
                            op1=mybir.AluOpType.add,
                        )
                        pend.append((kt, pt0, pt1))
                assert not pend and not pending_drain
                pending_drain = drain_stages(qb, ps_o)
            for f in pending_drain:
                f()
    nc.compile()
    return nc


_CACHED = {}


def _get_nc():
    if "nc" not in _CACHED:
        _CACHED["nc"] = build_nc()
    return _CACHED["nc"]


def _shard(Q, K, V):
    in_maps = []
    for c in range(NCORES):
        b = c // 4
        h0 = (c % 4) * HPC
        in_maps.append(
            {
                "q": np.ascontiguousarray(np.asarray(Q, np.float32)[b, h0 : h0 + HPC]),
                "k": np.ascontiguousarray(np.asarray(K, np.float32)[b, 0]),
                "v": np.ascontiguousarray(np.asarray(V, np.float32)[b, 0]),
            }
        )
    return in_maps


def kernel(Q, K, V, trace=False):
    nc = _get_nc()
    res = run_bass_kernel_spmd(nc, _shard(Q, K, V), list(range(NCORES)), trace=trace)
    _CACHED["last_result"] = res
    O = np.empty((B, H, S, D), np.float32)
    for c, r in enumerate(res.results):
        b = c // 4
        h0 = (c % 4) * HPC
        O[b, h0 : h0 + HPC] = r["o"]
    return O


# revision 20
# speedup vs baseline: 1.1157x; 1.1157x over previous
"""MQA attention kernel for Trainium2, 8-core SPMD (v7).

Problem: Q [2, 8, 2048, 64] fp32, K/V [2, 1, 2048, 64] fp32 (shared head).
out[b,h,q,:] = softmax(Q[b,h,q,:] @ K[b,0]^T / 8) @ V[b,0].

Sharding: 16 (b,h) pairs over 8 cores -> core c handles b = c//4,
heads 2*(c%4), 2*(c%4)+1 (both heads share one K/V slice).

Design notes:
  - "Permuted-s" staging: all inputs arrive as Xh[p, c, ...] = X[s=16p+c],
    fp16, via SWDGE cast-DMAs with 128 contiguous 4KB-read descriptors
    each (queries/keys/values all use the same permuted order, which is
    mathematically free; only the final output DMA un-permutes).
  - QT/KT [128, S] built by merged SBUF->SBUF XBAR transpose-DMAs
    (3D-out = chunked 128-partition transposes). K loads column-
    duplicated so K^T lands replicated on both partition halves.
  - MM1 (scores^T): per k-tile, two concurrent row-group matmuls
    (contract=64; h0 rows 0-63, h1 rows 64-127), each into its own
    single-bank PSUM tile. Score tiles rotate through 6 banks so the
    exp(kt) -> MM1(kt+3) pool dependency never binds the pipeline.
  - exp split: ACT true exp for h0; DVE Schraudolph exp for h1 in one
    tensor_scalar: int16(round(score*A+B)) bitcast fp16 ~= exp(score/8-C)
    (the -C shift cancels in softmax).
  - MM2: out^T[128, 512] += V_aug[kt]^T @ P^T; V_aug = [V | ones | 0-pad
    to 128 cols]. Col 64 accumulates the softmax denominator.
  - Drain (per qb, h): PSUM->SBUF fp16 copy rows 0:80 (ACT h0 / DVE h1),
    ONE merged XBAR transpose [80,512]->[128,4,80], DVE reciprocal of
    col 64, DVE tensor_scalar normalize -> fp32, un-permuting DMA out.
    Drain stages are EMITTED INTERLEAVED into the next qb's kt loop.
  - Main loop software-pipelined with MM2 trailing MM1 by 3 k-tiles so
    the PE FIFO never waits on exp. PSUM: 6 score banks + 2 out banks.
  - Two PE warmup phases bridge the staging window so HAM is at K=8/8
    when the main loop starts.
"""

import numpy as np

import concourse.bass as bass
import concourse.bacc as bacc
import concourse.mybir as mybir
import concourse.tile as tile
from concourse.bass_utils import run_bass_kernel_spmd

F32 = mybir.dt.float32
F16 = mybir.dt.float16
I16 = mybir.dt.int16

B, H, S, D = 2, 8, 2048, 64
HPC = 2            # heads per core
NCORES = 8
QB = 512           # query block (PSUM bank free-dim limit for fp32)
NQB = S // QB      # 4
KT_TILE = 128      # keys per k-tile (PE contract partition limit)
NKT = S // KT_TILE # 16
NC = NKT // NQB    # 4 c-chunks per qb block
MO = 80            # drained rows: 64 V + 1 denom + 15 pad (16 | 80)
VW = 128           # V_aug weight cols
SCALE = 1.0 / np.sqrt(np.float32(D))  # 0.125
C_SHIFT = 2.0      # exp(z - C_SHIFT): bounds p in fp16/int16; cancels in softmax

# Schraudolph fp16 exp: i16 = round(score*A2 + B2); bitcast fp16 ~= exp(score/8 - C)
A_FP16 = 1024.0 / np.log(2.0)              # 2^10 / ln 2
A2 = float(SCALE) * A_FP16                 # folded score scale
B2 = 15.0 * 1024.0 - C_SHIFT * A_FP16      # exponent bias - shift
MM2_SKEW = 2       # effective skew = MM2_SKEW + 1


def build_nc():
    nc = bacc.Bacc(None)
    Qd = nc.declare_dram_parameter("q", [HPC, S, D], F32, isOutput=False)
    Kd = nc.declare_dram_parameter("k", [S, D], F32, isOutput=False)
    Vd = nc.declare_dram_parameter("v", [S, D], F32, isOutput=False)
    Od = nc.declare_dram_parameter("o", [HPC, S, D], F32, isOutput=True)

    with tile.TileContext(nc) as tc:
        with (
            tc.tile_pool(name="const", bufs=1) as constp,
            tc.tile_pool(name="stage", bufs=1) as stp,
            tc.tile_pool(name="qk", bufs=1) as qkp,
            tc.tile_pool(name="vt", bufs=1) as vp,
            tc.tile_pool(name="pt", bufs=6) as ptp,
            tc.tile_pool(name="otF", bufs=2) as otFp,
            tc.tile_pool(name="otT", bufs=4) as otTp,
            tc.tile_pool(name="rec", bufs=4) as recp,
            tc.tile_pool(name="outsb", bufs=4) as outp,
            tc.tile_pool(name="psS", bufs=3, space="PSUM") as psSp,
            tc.tile_pool(name="psO", bufs=1, space="PSUM") as psOp,
        ):
            # Prime the exp table load so the ~2.7us ACT_TABLE_LOAD overlaps
            # the input DMA phase instead of stalling the first real exp.
            dummy = constp.tile([128, 64], F16)
            nc.vector.memset(dummy[:], 0.0)
            dummy32 = constp.tile([128, 16], F32)
            nc.vector.memset(dummy32[:], 0.0)
            nc.scalar.activation(
                dummy32[:], dummy32[:], mybir.ActivationFunctionType.Exp
            )

            # per-partition bias AP for the ACT exp (const-AP registry only
            # carries pre-registered values)
            bias_ap = constp.tile([128, 1], F32)
            nc.vector.memset(bias_ap[:], -float(C_SHIFT))

            # warmup output scribbles into qb0's accumulator (cleared by the
            # start=True MM2 later)
            ps_o0 = [psOp.tile([128, QB], F32, name=f"psO{h}") for h in range(HPC)]

            # PE warmup phase 1: unblocked, runs as soon as the PE is free.
            for _ in range(40):
                nc.tensor.matmul(
                    ps_o0[0][0:64, 0:64],
                    lhsT=dummy[:, 0:64],
                    rhs=dummy[:],
                    start=True,
                    stop=True,
                )

            # ---- input staging: SWDGE cast-DMAs straight to fp16 SBUF
            # (contiguous 4KB-read descriptors thanks to the permuted-s
            # layout), then interleaved merged XBAR transposes ----
            # K duplicated into both 64-col halves (the XBAR consumes 128-wide
            # input column tiles; the duplicate lands K^T on both partition
            # halves of KT).
            Kh = stp.tile([128, NKT, 2, D], F16, name="Kh")
            for r in range(2):
                nc.gpsimd.dma_start(
                    out=Kh[:, :, r, :],
                    in_=Kd.ap().rearrange("(p c) d -> p c d", p=128),
                )
            Qh = stp.tile([128, NKT, HPC, D], F16, name="Qh")
            for h in range(HPC):
                nc.gpsimd.dma_start(
                    out=Qh[:, :, h, :],
                    in_=Qd.ap()[h].rearrange("(p c) d -> p c d", p=128),
                )
            # V_aug [128k, c, 128] fp16: cols 0-63 = V (same permuted key
            # order as KT), col 64 = 1.0 (denominator), cols 65-127 = 0.
            Vt = vp.tile([128, NKT, VW], F16)
            nc.vector.memset(Vt[:, :, D + 1 : VW], 0.0)
            nc.vector.memset(Vt[:, :, D : D + 1], 1.0)
            nc.gpsimd.dma_start(
                out=Vt[:, :, 0:D],
                in_=Vd.ap().rearrange("(p c) d -> p c d", p=128),
            )

            # PE warmup phase 2: gated on the K load so it bridges the
            # staging window right up to the main loop.
            for _ in range(24):
                nc.tensor.matmul(
                    ps_o0[0][0:64, 0:128],
                    lhsT=dummy[:, 0:64],
                    rhs=Kh[:, 0, :, :].rearrange("p r d -> p (r d)"),
                    start=True,
                    stop=True,
                )

            # KT/QT [128, c, 128] fp16 via interleaved chunked XBAR
            # transposes: col (c, p) of KT/QT is key/query s = 16p + c.
            KT = qkp.tile([128, NKT, 128], F16, name="KT")
            QT = qkp.tile([128, NKT, 128], F16, name="QT")
            for c in range(NQB):
                cs = slice(c * NC, (c + 1) * NC)
                nc.sync.dma_start(
                    out=KT[:, cs, :],
                    in_=Kh[:, cs, :, :].rearrange("p c r d -> p (c r d)"),
                    transpose=True,
                )
                nc.sync.dma_start(
                    out=QT[:, cs, :],
                    in_=Qh[:, cs, :, :].rearrange("p c h d -> p (c h d)"),
                    transpose=True,
                )

            def QTs(h, qb):
                # [64, 512] moving operand for head h, query block qb
                return (
                    QT[64 * h : 64 * (h + 1), :, :]
                    .rearrange("p c s -> p (c s)")[:, qb * QB : (qb + 1) * QB]
                )

            def KTs(h, kt):
                return KT[64 * h : 64 * (h + 1), kt, :]

            # ---- main loop; the previous qb's drain is emitted interleaved
            # into this qb's kt loop ----
            def drain_stages(qb, ps_o):
                st = {"otF": [None, None], "otT": [None, None], "rec": [None, None],
                      "out": [None, None]}

                def s_copy(h):
                    def f():
                        st["otF"][h] = otFp.tile([MO, QB], F16, name=f"otF{h}")
                        if h == 0:
                            nc.scalar.copy(st["otF"][h][:], ps_o[h][0:MO, :])
                        else:
                            nc.vector.tensor_copy(st["otF"][h][:], ps_o[h][0:MO, :])
                    return f

                def s_transp(h):
                    def f():
                        st["otT"][h] = otTp.tile(
                            [128, NC, MO], F16, name=f"otT{h}"
                        )
                        nc.sync.dma_start(
                            out=st["otT"][h][:], in_=st["otF"][h][:], transpose=True
                        )
                    return f

                def s_recip(h):
                    def f():
                        otT = st["otT"][h]
                        rec = recp.tile([128, NC, 1], F32)
                        nc.vector.reciprocal(rec[:], otT[:, :, D : D + 1])
                        st["rec"][h] = rec
                        st["out"][h] = outp.tile([128, NC, D], F32, name=f"osb{h}")
                        for j in range(NC // 2):
                            nc.vector.tensor_scalar_mul(
                                st["out"][h][:, j, :], otT[:, j, 0:D], rec[:, j, :]
                            )
                    return f

                def s_norm(h):
                    def f():
                        otT, rec, outsb = st["otT"][h], st["rec"][h], st["out"][h]
                        for j in range(NC // 2, NC):
                            nc.vector.tensor_scalar_mul(
                                outsb[:, j, :], otT[:, j, 0:D], rec[:, j, :]
                            )
                        # un-permute: otT row pp, chunk j -> q = 16*pp + 4*qb + j
                        nc.sync.dma_start(
                            out=Od.ap()[h]
                            .rearrange("(p c) d -> p c d", p=128)[
                                :, qb * NC : (qb + 1) * NC, :
                            ],
                            in_=outsb[:],
                        )
                    return f

                return [
                    s_copy(0), s_copy(1), s_transp(0), s_transp(1),
                    s_recip(0), s_norm(0), s_recip(1), s_norm(1),
                ]

            pending_drain = []
            for qb in range(NQB):
                ps_o = (
                    ps_o0
                    if qb == 0
                    else [psOp.tile([128, QB], F32, name=f"psO{h}") for h in range(HPC)]
                )
                pend = []  # software pipeline: MM2 trails MM1 by MM2_SKEW+1 kts
                for kt in range(NKT + MM2_SKEW + 1):
                    if kt < NKT:
                        ps_s = [
                            psSp.tile([128, QB], F32, name=f"psS{h}")
                            for h in range(HPC)
                        ]
                        for h in range(HPC):
                            nc.tensor.matmul(
                                ps_s[h][:],
                                lhsT=KTs(h, kt),
                                rhs=QTs(h, qb),
                                start=True,
                                stop=True,
                            )
                    if len(pend) > (MM2_SKEW if kt < NKT else 0):
                        pkt, p0, p1 = pend.pop(0)
                        for h, rhs in ((0, p0[:]), (1, p1[:].bitcast(F16))):
                            nc.tensor.matmul(
                                ps_o[h][:],
                                lhsT=Vt[:, pkt, :],
                                rhs=rhs,
                                start=(pkt == 0),
                                stop=(pkt == NKT - 1),
                            )
                    if pending_drain and kt >= 1:
                        pending_drain.pop(0)()
                    if kt < NKT:
                        # exp: ACT (true) for h0, DVE (Schraudolph) for h1
                        pt0 = ptp.tile([128, QB], F16, name="pt0")
                        nc.scalar.activation(
                            pt0[:],
                            ps_s[0][:],
                            mybir.ActivationFunctionType.Exp,
                            scale=float(SCALE),
                            bias=bias_ap[:],
                        )
                        pt1 = ptp.tile([128, QB], I16, name="pt1")
                        nc.vector.tensor_scalar(
                            pt1[:],
                            ps_s[1][:],
                            float(A2),
                            float(B2),
                            op0=mybir.AluOpType.mult,
                            op1=mybir.AluOpType.add,
                        )
                        pend.append((kt, pt0, pt1))
                assert not pend and not pending_drain
                pending_drain = drain_stages(qb, ps_o)
            for f in pending_drain:
                f()
    nc.compile()
    return nc


_CACHED = {}


def _get_nc():
    if "nc" not in _CACHED:
        _CACHED["nc"] = build_nc()
    return _CACHED["nc"]


def _shard(Q, K, V):
    in_maps = []
    for c in range(NCORES):
        b = c // 4
        h0 = (c % 4) * HPC
        in_maps.append(
            {
                "q": np.ascontiguousarray(np.asarray(Q, np.float32)[b, h0 : h0 + HPC]),
                "k": np.ascontiguousarray(np.asarray(K, np.float32)[b, 0]),
                "v": np.ascontiguousarray(np.asarray(V, np.float32)[b, 0]),
            }
        )
    return in_maps


def kernel(Q, K, V, trace=False):
    nc = _get_nc()
    res = run_bass_kernel_spmd(nc, _shard(Q, K, V), list(range(NCORES)), trace=trace)
    _CACHED["last_result"] = res
    O = np.empty((B, H, S, D), np.float32)
    for c, r in enumerate(res.results):
        b = c // 4
        h0 = (c % 4) * HPC
        O[b, h0 : h0 + HPC] = r["o"]
    return O
